# revision 13
# baseline (speedup 1.0000x reference)
"""Trainium2 Bass kernel for the cross-attention gating layer.

Computes, for x1 = input1[0] [S, src], x2 = input2[0] [T, tgt]:
    Q = x1 @ w_q.T; K = x2 @ w_k.T; V = x2 @ w_v.T
    attn = softmax(Q @ K.T / sqrt(128), axis=1)
    out  = (attn @ V) @ w_output.T          # [S, 1] gate
    res  = x1 * (1 - out)                   # (x1 if sum(x2)==0)

Key algebraic simplification: (attn @ V) @ w_output.T == attn @ (V @ w_output.T),
so V collapses into a single vector projection vg = x2 @ (w_output @ w_v).T [T].
Per query row s:  gate[s] = sum_t e[t,s]*vg[t] / sum_t e[t,s],
with e[t,s] = exp(scores[s,t]/sqrt(128)).

Sharding: the key/context dim T is sharded across the 8 NeuronCores; every
core streams the FULL query sequence through its own T-shard, accumulating
unnormalized (numerator, denominator) partial sums for every query. A single
tiny (64KB) ReduceScatter(add) at the very end combines the partials and
hands each core the q-shard it writes out — no mid-kernel gather of K/V, so
the slow collective path is off the critical path entirely (a 16-byte dummy
AllGather at kernel start warms the ncfw machinery in the shadow of the
prologue).

Everything runs in the transposed orientation (scores^T = K_local @ Q^T),
which makes every matmul operand land in its natural layout: no big
transposes anywhere, and the softmax denominator falls out of a ones-column
interleaved into the attn@vg stationary operand. The two accumulating
attn@vg matmuls of each t-tile pair are packed onto disjoint PE column
groups (tile_position) so they run concurrently.
"""

import sys

sys.path.insert(0, "/opt/trn_rl_repo")

import numpy as np
import ml_dtypes

BF16 = ml_dtypes.bfloat16

NCORES = 8
S = 8192
T = 8192
SRC = 768
DK = 128
SL = S // NCORES  # 1024 rows in this core's OUTPUT q-shard
TL = T // NCORES  # 1024 local key rows
FC = SRC // 128  # 6 feature chunks
NQT = SL // 128  # 8 q tiles per q-chunk
NTT = TL // 128  # 8 local t tiles
NQC = S // SL  # 8 q chunks streamed through every core

_CACHE = {}


def _build_nc():
    import concourse.bacc as bacc
    import concourse.mybir as mybir
    import concourse.tile as tile
    from concourse import masks

    dt = mybir.dt
    fp32 = dt.float32
    bf16 = dt.bfloat16

    nc = bacc.Bacc("TRN2", target_bir_lowering=False, debug=False, num_devices=NCORES)

    # I/O (all host-side pre-swizzled to partition-major contiguous layouts)
    x1f = nc.dram_tensor("x1f", [128, NQT, SRC], fp32, kind="ExternalInput").ap()
    x1t = nc.dram_tensor("x1t", [NQC, 128, FC * SL], bf16, kind="ExternalInput").ap()
    x2t = nc.dram_tensor("x2t", [128, FC, TL], bf16, kind="ExternalInput").ap()
    wq = nc.dram_tensor("wq", [128, FC, DK], bf16, kind="ExternalInput").ap()
    wk = nc.dram_tensor("wk", [128, FC, DK], bf16, kind="ExternalInput").ap()
    wvo = nc.dram_tensor("wvo", [128, FC], bf16, kind="ExternalInput").ap()
    outp = nc.dram_tensor("outp", [128, NQT, SRC], fp32, kind="ExternalOutput").ap()

    rs_in = nc.dram_tensor("rs_in", [NCORES * 2 * SL], fp32).ap()
    rs_out = nc.dram_tensor("rs_out", [2 * SL], fp32).ap()
    din = nc.dram_tensor("din", [8], bf16).ap()
    dout = nc.dram_tensor("dout", [NCORES, 8], bf16, addr_space="Shared").ap()

    ISCALE = float(1.0 / np.sqrt(np.float32(128.0)))

    with tile.TileContext(nc) as tc:
        with (
            tc.tile_pool(name="const", bufs=1) as const,
            tc.tile_pool(name="work", bufs=1) as work,
            tc.tile_pool(name="xt1c", bufs=3) as xt1p,
            tc.tile_pool(name="qtc", bufs=2) as qtp,
            tc.tile_pool(name="pt", bufs=6) as ptp,
            tc.tile_pool(name="gp", bufs=2) as gpp,
        ):
            # ---- dummy collective warms the ncfw/TOPSP pipeline in the
            # shadow of the prologue (nothing waits on it) -----------------
            dtile = work.tile([1, 8], bf16)
            nc.vector.memset(dtile[:], 0.0)
            dexp = work.tile([1, 8], bf16)
            nc.scalar.activation(
                dexp[:], dtile[:], mybir.ActivationFunctionType.Exp, scale=1.0
            )
            nc.gpsimd.dma_start(din.rearrange("(o j) -> o j", o=1), dtile[:])
            nc.gpsimd.collective_compute(
                "AllGather",
                mybir.AluOpType.bypass,
                replica_groups=[list(range(NCORES))],
                ins=[din[:]],
                outs=[dout[:]],
            )

            # ---- loads (xt2 leads the sync queue: it gates the K/vg
            # projections; everything else rides the idle SWDGE path) ------
            xt2 = const.tile([128, FC * TL], bf16)
            nc.sync.dma_start(xt2[:], x2t.rearrange("p c j -> p (c j)"))
            wk_s = const.tile([128, FC * DK], bf16)
            nc.gpsimd.dma_start(wk_s[:], wk.rearrange("p c m -> p (c m)"))
            wvo_s = const.tile([128, FC], bf16)
            nc.gpsimd.dma_start(wvo_s[:], wvo[:])
            wq_s = const.tile([128, FC * DK], bf16)
            nc.gpsimd.dma_start(wq_s[:], wq.rearrange("p c m -> p (c m)"))
            x1f_s = const.tile([128, NQT * SRC], fp32)

            ident = const.tile([128, 128], fp32)
            masks.make_identity(nc, ident[:])
            ident_bf = const.tile([1, 1], bf16)
            nc.vector.memset(ident_bf[:], 1.0)

            # ---- local projections: K^T [dk, TL] and vg [TL] -------------
            ppsum_cm = tc.tile_pool(name="ppsum", bufs=1, space="PSUM")
            ppsum = ppsum_cm.__enter__()
            kps = ppsum.tile([128, TL], fp32)
            vgps = ppsum.tile([1, TL], fp32)
            for h in range(2):
                qh = slice(512 * h, 512 * h + 512)
                for c in range(FC):
                    xs = slice(TL * c + 512 * h, TL * c + 512 * h + 512)
                    nc.tensor.matmul(
                        kps[:, qh],
                        wk_s[:, DK * c : DK * (c + 1)],
                        xt2[:, xs],
                        start=(c == 0),
                        stop=(c == FC - 1),
                    )
                for c in range(FC):
                    xs = slice(TL * c + 512 * h, TL * c + 512 * h + 512)
                    nc.tensor.matmul(
                        vgps[:, qh],
                        wvo_s[:, c : c + 1],
                        xt2[:, xs],
                        start=(c == 0),
                        stop=(c == FC - 1),
                    )
            kt_s = const.tile([128, TL], bf16)
            nc.vector.tensor_copy(kt_s[:], kps[:])
            vg_row = work.tile([1, TL], bf16)
            nc.vector.tensor_copy(vg_row[:], vgps[:])

            # vg as per-t-tile lhsT columns interleaved with a ones column,
            # via 8 tiny PE transposes ([1,128] -> [128,1])
            vgtr_cm = tc.tile_pool(name="vgtr", bufs=2, space="PSUM")
            vgtrp = vgtr_cm.__enter__()
            vgi = const.tile([128, 2 * NTT], bf16)
            nc.vector.memset(vgi[:], 1.0)
            for t in range(NTT):
                trv = vgtrp.tile([128, 1], bf16)
                nc.tensor.transpose(
                    trv[:], vg_row[0:1, 128 * t : 128 * (t + 1)], ident_bf[0:1, 0:1]
                )
                nc.vector.tensor_copy(vgi[:, 2 * t : 2 * t + 1], trv[:])
            vgtr_cm.__exit__(None, None, None)
            ppsum_cm.__exit__(None, None, None)

            # ---- stream the full query sequence through the local shard --
            qproj_cm = tc.tile_pool(name="qproj", bufs=1, space="PSUM")
            qproj = qproj_cm.__enter__()
            scps_cm = tc.tile_pool(name="scps", bufs=2, space="PSUM")
            scps = scps_cm.__enter__()
            avpsp_cm = tc.tile_pool(name="avps", bufs=1, space="PSUM")
            avpsp = avpsp_cm.__enter__()

            gps = []
            for qc in range(NQC):
                # load x1^T chunk and project Q^T for these 1024 queries
                xt1c = xt1p.tile([128, FC * SL], bf16)
                nc.gpsimd.dma_start(xt1c[:], x1t[qc])
                qps = qproj.tile([128, SL], fp32)
                for h in range(2):
                    qh = slice(512 * h, 512 * h + 512)
                    for c in range(FC):
                        xs = slice(SL * c + 512 * h, SL * c + 512 * h + 512)
                        nc.tensor.matmul(
                            qps[:, qh],
                            wq_s[:, DK * c : DK * (c + 1)],
                            xt1c[:, xs],
                            start=(c == 0),
                            stop=(c == FC - 1),
                        )
                qT = qtp.tile([128, SL], bf16)
                nc.vector.tensor_copy(qT[:], qps[:])

                # previous chunk's partial combine + store (off critical
                # path): the two column-group partials are summed by the
                # SDMA inline CCE via an accumulating second DMA
                if qc > 0:
                    pg = gps[qc - 1]
                    dst = rs_in[2 * SL * (qc - 1) : 2 * SL * qc].rearrange(
                        "(o j) -> o j", o=2, j=SL
                    )
                    nc.gpsimd.dma_start(dst, pg[0:2, :])
                    for g in range(1, 4):
                        nc.gpsimd.dma_start(
                            dst, pg[32 * g : 32 * g + 2, :],
                            accum_op=mybir.AluOpType.add,
                        )

                avps = avpsp.tile([128, SL], fp32)
                for tp in range(NTT // 4):
                    pts = []
                    for g in range(4):
                        tt = 4 * tp + g
                        sps = scps.tile([128, SL], fp32)
                        for h in range(2):
                            qh = slice(512 * h, 512 * h + 512)
                            nc.tensor.matmul(
                                sps[:, qh],
                                kt_s[:, 128 * tt : 128 * (tt + 1)],
                                qT[:, qh],
                                start=True,
                                stop=True,
                            )
                        pT = ptp.tile([128, SL], bf16)
                        nc.scalar.activation(
                            pT[:],
                            sps[:],
                            mybir.ActivationFunctionType.Exp,
                            scale=ISCALE,
                        )
                        pts.append(pT)
                    # the four accumulating matmuls of a quad land on the four
                    # disjoint PE column groups and run concurrently
                    for h in range(2):
                        qh = slice(512 * h, 512 * h + 512)
                        for g in range(4):
                            tt = 4 * tp + g
                            nc.tensor.matmul(
                                avps[32 * g : 32 * g + 2, qh],
                                vgi[:, 2 * tt : 2 * tt + 2],
                                pts[g][:, qh],
                                start=(tp == 0),
                                stop=(tp == NTT // 4 - 1),
                                tile_position=(0, 32 * g),
                            )

                # single copy releases the accumulator banks quickly; the
                # partial combine happens early next iteration
                gp = gpp.tile([98, SL], fp32)
                nc.vector.tensor_copy(gp[:], avps[0:98, :])
                gps.append(gp)

            pg = gps[NQC - 1]
            dst = rs_in[2 * SL * (NQC - 1) : 2 * SL * NQC].rearrange(
                "(o j) -> o j", o=2, j=SL
            )
            nc.gpsimd.dma_start(dst, pg[0:2, :])
            for g in range(1, 4):
                nc.gpsimd.dma_start(
                    dst, pg[32 * g : 32 * g + 2, :], accum_op=mybir.AluOpType.add
                )
            # epilogue-only input, loaded once SWDGE is quiet
            nc.gpsimd.dma_start(x1f_s[:], x1f.rearrange("p q f -> p (q f)"))

            avpsp_cm.__exit__(None, None, None)
            scps_cm.__exit__(None, None, None)
            qproj_cm.__exit__(None, None, None)

            # ---- combine partials across cores; core i keeps q-shard i ---
            nc.gpsimd.collective_compute(
                "ReduceScatter",
                mybir.AluOpType.add,
                replica_groups=[list(range(NCORES))],
                ins=[rs_in[:]],
                outs=[rs_out[:]],
            )
            go_s = work.tile([2, SL], fp32)
            nc.sync.dma_start(
                go_s[:], rs_out.rearrange("(o j) -> o j", o=2, j=SL)
            )

            # ---- gate + output ------------------------------------------
            trps_cm = tc.tile_pool(name="trps", bufs=2, space="PSUM")
            trps = trps_cm.__enter__()
            gq = work.tile([128, 2 * NQT], fp32)
            for q in range(NQT):
                trp = trps.tile([128, 2], fp32)
                nc.tensor.transpose(
                    trp[:], go_s[:, 128 * q : 128 * (q + 1)], ident[0:2, 0:2]
                )
                nc.vector.tensor_copy(gq[:, 2 * q : 2 * q + 2], trp[:])
            trps_cm.__exit__(None, None, None)

            recip = work.tile([128, NQT], fp32)
            nc.vector.reciprocal(recip[:], gq[:, 1 : 2 * NQT : 2])
            onem = work.tile([128, NQT], fp32)
            nc.vector.tensor_tensor(
                onem[:], gq[:, 0 : 2 * NQT : 2], recip[:], mybir.AluOpType.mult
            )
            nc.vector.tensor_scalar(
                onem[:], onem[:], -1.0, 1.0, mybir.AluOpType.mult, mybir.AluOpType.add
            )

            for q in range(NQT):
                og = gpp.tile([128, SRC], fp32, tag="og")
                nc.vector.tensor_scalar(
                    og[:],
                    x1f_s[:, SRC * q : SRC * (q + 1)],
                    onem[:, q : q + 1],
                    None,
                    mybir.AluOpType.mult,
                )
                eng = nc.sync if q % 2 == 0 else nc.gpsimd
                eng.dma_start(outp[:, q, :], og[:])

    nc.compile()
    return nc


def _get_nc():
    if "nc" not in _CACHE:
        _CACHE["nc"] = _build_nc()
    return _CACHE["nc"]


def build_in_maps(input1, input2, w_q, w_k, w_v, w_output):
    x1 = np.asarray(input1, dtype=np.float32)[0]
    x2 = np.asarray(input2, dtype=np.float32)[0]

    wq_sw = np.ascontiguousarray(
        np.asarray(w_q, np.float32).T.astype(BF16).reshape(FC, 128, DK).transpose(1, 0, 2)
    )
    wk_sw = np.ascontiguousarray(
        np.asarray(w_k, np.float32).T.astype(BF16).reshape(FC, 128, DK).transpose(1, 0, 2)
    )
    wvo = (np.asarray(w_output, np.float32) @ np.asarray(w_v, np.float32))[0]
    wvo_sw = np.ascontiguousarray(wvo.astype(BF16).reshape(FC, 128).T)

    # full x1^T, chunked+partition-major: [NQC, 128, FC*SL]; shared by all cores
    x1t_full = np.ascontiguousarray(
        x1.astype(BF16)
        .T.reshape(FC, 128, NQC, SL)
        .transpose(2, 1, 0, 3)
        .reshape(NQC, 128, FC * SL)
    )

    in_maps = []
    for i in range(NCORES):
        sl = slice(i * SL, (i + 1) * SL)
        x1_sh = x1[sl]
        x2_sh = x2[sl]
        in_maps.append(
            {
                "x1f": np.ascontiguousarray(
                    x1_sh.reshape(NQT, 128, SRC).transpose(1, 0, 2)
                ),
                "x1t": x1t_full,
                "x2t": np.ascontiguousarray(
                    x2_sh.astype(BF16).T.reshape(FC, 128, TL).transpose(1, 0, 2)
                ),
                "wq": wq_sw,
                "wk": wk_sw,
                "wvo": wvo_sw,
            }
        )
    return in_maps


def kernel(input1, input2, w_q, w_k, w_v, w_output):
    from concourse.bass_utils import run_bass_kernel_spmd

    x1 = np.asarray(input1, dtype=np.float32)[0]
    x2 = np.asarray(input2, dtype=np.float32)[0]

    # Early-exit branch of the reference module.
    if x2.sum() == 0:
        return x1.copy()

    in_maps = build_in_maps(input1, input2, w_q, w_k, w_v, w_output)
    nc = _get_nc()
    res = run_bass_kernel_spmd(nc, in_maps, list(range(NCORES)))

    out = np.empty((S, SRC), dtype=np.float32)
    for i in range(NCORES):
        o = res.results[i]["outp"]  # [128, NQT, SRC]
        out[i * SL : (i + 1) * SL] = o.transpose(1, 0, 2).reshape(SL, SRC)
    return out
